# revision 57
# baseline (speedup 1.0000x reference)
"""Trainium2 Bass kernel for nn_Net_6maxFull (batch of tiny LSTM chains).

V2.6 design (sigma formulation, 10-slot schedule, 8-way interleave, FD=1024):
  - 30 LSTM cells in 10 slots x 3 cells (gen chain + 2 opp lanes/slot;
    opp chains may skip slots -- h carried via rhs rows + late copies).
  - Gates matmul M-layout 32-aligned [ai|af|2ag|ao] (parity-swapped o/g
    for odd interleave lanes); ONE sigmoid act gives S = [i, f, g', o]
    with g' = sigma(2ag) = (tanh(ag)+1)/2.
  - Z = [g'(DVE copy) | c(DMA)]; U = S[0:64] * Z  (one 2x-mode TT) gives
    [P=i.g', FC=f.c]; 2c2 = 4P + 2FC - 2i via two accumulating matmuls
    (U with coeffs 4,2; S[0:32] with -2I).
  - act2 = Tanh(scale=0.5) over a PAIR-shared psum [2c2_odd | 2c2_even],
    output shifted +64 so each T2 lands at its tile's o base (64/96);
    h = T2 * o as a plain TT, alternating DVE / GpSimd(Pool).
  - Head concat: 1 DMA per slot into CC tiles [120 rows = 4 slots x 30];
    heads contract CC directly with rearranged weights; F2/out packed x3.
  - DMA routing: st/ct bulk loads on SP (HWDGE); concat/late copies on
    gpsimd (SWDGE cheap triggers).
  - DVE lane rules honored: elementwise operand pairs share a 32-aligned
    base partition and never straddle the 64-lane boundary.
"""
import sys
import numpy as np

sys.path.insert(0, "/opt/trn_rl_repo")

B = 131072
NCORE = 8
BC = B // NCORE
H = 10

# slot schedule: cell = ("g", layer) or ("o", branch, step)
SLOTS = [
    [("g", 0), ("o", 0, 0), ("o", 1, 0)],
    [("g", 1), ("o", 0, 1), ("o", 1, 1)],
    [("g", 2), ("o", 0, 2), ("o", 1, 2)],
    [("g", 3), ("o", 0, 3), ("o", 2, 0)],
    [("g", 4), ("o", 1, 3), ("o", 2, 1)],
    [("g", 5), ("o", 2, 2), ("o", 3, 0)],
    [("g", 6), ("o", 2, 3), ("o", 4, 0)],
    [("g", 7), ("o", 3, 1), ("o", 4, 1)],
    [("g", 8), ("o", 3, 2), ("o", 4, 2)],
    [("g", 9), ("o", 3, 3), ("o", 4, 3)],
]
NSLOT = len(SLOTS)
W3C = 30          # rows per slot of h output (3 cells x 10)


def _is_start(cell):
    return (cell[0] == "g" and cell[1] == 0) or (cell[0] == "o" and cell[2] == 0)


def _pred(cell):
    return ("g", cell[1] - 1) if cell[0] == "g" else ("o", cell[1], cell[2] - 1)


def _x_rows(cell):
    if cell[0] == "g":
        return (0, 12)
    p = cell[1]
    s = 12 + 5 * p + 1
    return (s, s + 4)


class Plan:
    """Host-side layout plan for rhs rows / lhsT maps / late copies."""

    def __init__(self):
        self.slot = []
        # for each slot, where did each cell's h go (slot_idx, row) --
        # h of slot t cell k lives at rhs[t+1] rows 10k:10k+10
        pos_in = {}      # cell -> (slot, pos)
        for t, cells in enumerate(SLOTS):
            for k, c in enumerate(cells):
                pos_in[c] = (t, k)
        self.pos_in = pos_in
        # late-copy edges: pred h not in immediately preceding slot
        # (src_rhs_tile = pred_slot+1, rows 10*pred_pos; dst rhs[t][30:40])
        self.late = {}   # t -> (src_slot+1, src_row)
        for t, cells in enumerate(SLOTS):
            for c in cells:
                if _is_start(c):
                    continue
                pt_, pk = pos_in[_pred(c)]
                if pt_ != t - 1:
                    assert t not in self.late, "only one late edge per slot"
                    self.late[t] = (pt_ + 1, 10 * pk)
        # rhs layout per slot:
        #   [chain h 0:30 | pad | ct 32:64 | x rows | h-state rows | late?]
        # (t=0: x rows sit at 0:20 instead of chain rows). ct rows are part
        # of the gates contraction with zero weights; after the gates matmul
        # the rows 0:32 are overwritten by the g' copy so the merged TT can
        # read [g' | c] from one tile at base 0.
        for t, cells in enumerate(SLOTS):
            info = {"cells": cells}
            rows = []        # (kind, cell): x rows for start cells
            for c in cells:
                if _is_start(c):
                    rows.append(("x", c))
            for c in cells:
                rows.append(("h", c))
            info["strows"] = rows
            nx = sum(12 if c[0] == "g" else 4 for k, c in rows if k == "x")
            info["nx"] = nx
            if t == 0:
                info["xoff"] = 0              # x rows at 0:nx
                info["hoff"] = 64             # h-state rows
                info["st_lo"] = 0
            else:
                info["xoff"] = 64
                info["hoff"] = 64 + nx
                info["st_lo"] = 30
            info["late_off"] = info["hoff"] + W3C if t in self.late else None
            info["Kmm"] = info["hoff"] + W3C + (10 if t in self.late else 0)
            info["st_hi"] = info["hoff"] + W3C   # st dma covers [st_lo, st_hi)
            assert info["Kmm"] <= 128
            self.slot.append(info)

    def chain_row(self, t, cell):
        """rhs[t] row where this chained cell's input h lives."""
        pt_, pk = self.pos_in[_pred(cell)]
        if pt_ == t - 1:
            return 10 * pk
        return self.slot[t]["late_off"]


PLAN = Plan()


def _cell_w(inp, cell):
    if cell[0] == "g":
        i = cell[1]
        if i == 0:
            return (inp["W_g0_ih"], inp["W_g0_hh"], inp["b_g0_ih"] + inp["b_g0_hh"])
        return (inp["W_g_ih"][i - 1], inp["W_g_hh"][i - 1],
                inp["b_g_ih"][i - 1] + inp["b_g_hh"][i - 1])
    p, s = cell[1], cell[2]
    if s == 0:
        return (inp["W_o0_ih"][p], inp["W_o0_hh"][p],
                inp["b_o0_ih"][p] + inp["b_o0_hh"][p])
    return (inp["W_o_ih"][p][s - 1], inp["W_o_hh"][p][s - 1],
            inp["b_o_ih"][p][s - 1] + inp["b_o_hh"][p][s - 1])


# CC tile mapping: slot t -> (cc_idx, row_base)
def _cc_of(t):
    return t // 4, 30 * (t % 4)


def pack_host(inp, np_dt):
    """Build all DRAM-side arrays (full batch; shard columns later)."""
    f32 = np.float32
    out = {}
    Bt = inp["x"].shape[0]
    xT = np.ascontiguousarray(np.asarray(inp["x"], f32).T.astype(np_dt))   # [37,B]

    def state(cell):
        if cell[0] == "g":
            return (np.asarray(inp["gen_h"][cell[1]], f32).T,
                    np.asarray(inp["gen_c"][cell[1]], f32).T)
        return (np.asarray(inp["opp_h"][cell[1]][cell[2]], f32).T,
                np.asarray(inp["opp_c"][cell[1]][cell[2]], f32).T)

    # gate group slices in torch order i,f,g,o within the [40, din] weights
    GS = {"i": slice(0, 10), "f": slice(10, 20), "g": slice(20, 30), "o": slice(30, 40)}
    # two M-layout parities: even tiles o@96 (g'@64), odd tiles o@64 (g'@96)
    GCOLS = ({"i": 0, "f": 32, "g": 64, "o": 96},
             {"i": 0, "f": 32, "o": 64, "g": 96})
    GSC = {"i": 1.0, "f": 1.0, "g": 2.0, "o": 1.0}

    for t, info in enumerate(PLAN.slot):
        cells = info["cells"]
        # ---- st block rows [st_lo, st_hi): pad/ct/x/h-state interleaved ----
        st = np.zeros((info["st_hi"] - info["st_lo"], Bt), np_dt)

        def put(row, data):
            st[row - info["st_lo"]:row - info["st_lo"] + data.shape[0]] = data

        for k, c in enumerate(cells):
            h0, c0 = state(c)
            put(32 + 10 * k, c0.astype(np_dt))           # ct rows 32:64
            put(info["hoff"] + 10 * k, h0.astype(np_dt))
        xrow_of = {}
        r = info["xoff"]
        for kind, c in info["strows"]:
            if kind == "x":
                a, b = _x_rows(c)
                put(r, xT[a:b])
                xrow_of[c] = r
                r += b - a
        out[f"st{t}"] = st

        # ---- gates lhsT [Kmm, 128] + bias1 [128,1], per parity ----
        for par in (0, 1):
            GCOL = GCOLS[par]
            lw = np.zeros((info["Kmm"], 128), f32)
            b1 = np.zeros((128, 1), f32)
            for k, c in enumerate(cells):
                Wih, Whh, bvec = (np.asarray(a, f32) for a in _cell_w(inp, c))
                if _is_start(c):
                    r0 = xrow_of[c]
                    din = Wih.shape[1]
                    for gn in "ifgo":
                        lw[r0:r0 + din, GCOL[gn] + 10 * k:GCOL[gn] + 10 * k + 10] = \
                            GSC[gn] * Wih[GS[gn]].T
                else:
                    r0 = PLAN.chain_row(t, c)
                    for gn in "ifgo":
                        lw[r0:r0 + 10, GCOL[gn] + 10 * k:GCOL[gn] + 10 * k + 10] = \
                            GSC[gn] * Wih[GS[gn]].T
                r0 = info["hoff"] + 10 * k
                for gn in "ifgo":
                    lw[r0:r0 + 10, GCOL[gn] + 10 * k:GCOL[gn] + 10 * k + 10] = \
                        GSC[gn] * Whh[GS[gn]].T
                    b1[GCOL[gn] + 10 * k:GCOL[gn] + 10 * k + 10, 0] = GSC[gn] * bvec[GS[gn]]
            out[f"lwg{t}_{par}"] = lw.astype(np_dt)
            out[f"bias{t}_{par}"] = b1

    # ---- iadd lhsT [96,32]: 2c2 = 4*P + 2*FC - 2*i over U=[P|FC|icopy] ----
    ia = np.zeros((96, 32), f32)
    for j in range(W3C):
        ia[j, j] = 4.0
        ia[32 + j, j] = 2.0
        ia[64 + j, j] = -2.0
    out["iadda"] = ia.astype(np_dt)

    # ---- heads: contract CC tiles [120 rows = 4 slots x (g|opp|opp)] ----
    W1 = np.asarray(inp["W1"], f32)      # [50, 100]
    W1o = np.asarray(inp["W1o"], f32)    # [20, 40]
    W2 = np.asarray(inp["W2"], f32)      # [10, 70]
    W3 = np.asarray(inp["W3"], f32)      # [1, 10]
    # wh1: -> [F1(0:50) | zA(64:84) | zB(84:104)]; wh2: -> [zC|zD|zE] (60)
    for q in range(3):
        rows = 120 if q < 2 else 60
        w1q = np.zeros((rows, 104), f32)
        w2q = np.zeros((rows, 60), f32)
        for t in range(4 * q, min(4 * q + 4, NSLOT)):
            rb = 30 * (t % 4)
            for k, c in enumerate(SLOTS[t]):
                rr = rb + 10 * k
                if c[0] == "g":
                    w1q[rr:rr + 10, 0:50] = W1[:, 10 * c[1]:10 * c[1] + 10].T
                else:
                    p, s = c[1], c[2]
                    blk = W1o[:, 10 * s:10 * s + 10].T
                    if p < 2:
                        w1q[rr:rr + 10, 64 + 20 * p:64 + 20 * p + 20] = blk
                    else:
                        w2q[rr:rr + 10, 20 * (p - 2):20 * (p - 2) + 20] = blk
        out[f"wh1_{q}"] = w1q.astype(np_dt)
        out[f"wh2_{q}"] = w2q.astype(np_dt)
    w2o = (W2[:, 50:70] / 5.0).T                      # [20, 10]
    # 32 output cols (10 real + 22 zero) so the packed psF psum rows
    # 32j..32j+32 are all matmul-written (no stale-garbage rows feeding
    # the later block-diag p3 contraction).
    w2full = np.zeros((104, 32), f32)
    w2full[0:50, 0:10] = W2[:, 0:50].T
    w2full[64:104, 0:10] = np.vstack([w2o, w2o])
    out["w2full"] = w2full.astype(np_dt)
    w2cde = np.zeros((60, 32), f32)
    w2cde[:, 0:10] = np.vstack([w2o, w2o, w2o])
    out["w2cde"] = w2cde.astype(np_dt)
    w3blk = np.zeros((74, 3), f32)                    # block-diag W3 x3 tiles
    for j in range(3):
        w3blk[32 * j:32 * j + 10, j] = np.asarray(W3, f32)[0]
    out["w3blk"] = w3blk.astype(np_dt)
    hb = np.zeros((128, 4), f32)
    hb[0:50, 0] = np.asarray(inp["b1"], f32)
    hb[64:104, 0] = np.tile(np.asarray(inp["b1o"], f32), 2)
    hb[0:60, 1] = np.tile(np.asarray(inp["b1o"], f32), 3)
    for j in range(3):                                     # packed x3 at 32j
        hb[32 * j:32 * j + 10, 2] = np.asarray(inp["b2"], f32)
    hb[0:3, 3] = float(np.asarray(inp["b3"], f32)[0])
    out["hbias"] = hb
    return out


def build_nc(Bc, FD, np_dt, n_ilv=8):
    """SPMD Bass program for one core; n_ilv batch tiles interleaved."""
    import concourse.bass as bass
    import concourse.tile as tile
    from concourse import bacc, mybir

    dt = {np.dtype(np.float32): mybir.dt.float32}.get(np.dtype(np_dt))
    if dt is None:
        import ml_dtypes
        assert np.dtype(np_dt) == np.dtype(ml_dtypes.bfloat16)
        dt = mybir.dt.bfloat16
    f32 = mybir.dt.float32
    AF = mybir.ActivationFunctionType
    ALU = mybir.AluOpType

    NMM = 512                       # psum f32 bank cols
    n_tiles = Bc // FD
    assert Bc % FD == 0 and FD % NMM == 0
    nchunk = FD // NMM
    assert n_tiles % n_ilv == 0
    # act2/head pack groups (PE out base partition must be 0/32/64 -> max 3)
    GROUPS = [list(range(g, min(g + 3, n_ilv))) for g in range(0, n_ilv, 3)]

    nc = bacc.Bacc(None, target_bir_lowering=False, debug=False)
    P = PLAN.slot
    dr = {}
    for t in range(NSLOT):
        dr[f"st{t}"] = nc.declare_dram_parameter(f"st{t}", [P[t]["st_hi"] - P[t]["st_lo"], Bc], dt, isOutput=False)
        for par in (0, 1):
            dr[f"lwg{t}_{par}"] = nc.declare_dram_parameter(f"lwg{t}_{par}", [P[t]["Kmm"], 128], dt, isOutput=False)
            dr[f"bias{t}_{par}"] = nc.declare_dram_parameter(f"bias{t}_{par}", [128, 1], f32, isOutput=False)
    dr["iadda"] = nc.declare_dram_parameter("iadda", [96, 32], dt, isOutput=False)
    for q in range(3):
        rows = 120 if q < 2 else 60
        dr[f"wh1_{q}"] = nc.declare_dram_parameter(f"wh1_{q}", [rows, 104], dt, isOutput=False)
        dr[f"wh2_{q}"] = nc.declare_dram_parameter(f"wh2_{q}", [rows, 60], dt, isOutput=False)
    for name, shp in [("w2full", [104, 32]), ("w2cde", [60, 32]), ("w3blk", [74, 3])]:
        dr[name] = nc.declare_dram_parameter(name, shp, dt, isOutput=False)
    dr["hbias"] = nc.declare_dram_parameter("hbias", [128, 4], f32, isOutput=False)
    out_d = nc.declare_dram_parameter("out", [1, Bc], f32, isOutput=True)

    from contextlib import ExitStack
    with tile.TileContext(nc) as tc:
        with ExitStack() as ctx:
            consts = ctx.enter_context(tc.tile_pool(name="consts", bufs=1))
            rhsp = ctx.enter_context(tc.tile_pool(name="rhs", bufs=2))
            sp = ctx.enter_context(tc.tile_pool(name="sS", bufs=1))
            up = ctx.enter_context(tc.tile_pool(name="uU", bufs=1))
            s2p = ctx.enter_context(tc.tile_pool(name="s2", bufs=2))
            ccp = ctx.enter_context(tc.tile_pool(name="cc", bufs=1))
            fhp = ctx.enter_context(tc.tile_pool(name="fh", bufs=2))
            outp = ctx.enter_context(tc.tile_pool(name="osb", bufs=1))
            pg = ctx.enter_context(tc.tile_pool(name="pgate", bufs=4, space="PSUM"))

            # ---- constants ----
            lwg, bias = {}, {}
            for t in range(NSLOT):
                for par in (0, 1):
                    key = (t, par)
                    lwg[key] = consts.tile([P[t]["Kmm"], 128], dt,
                                           tag=f"lwg{t}_{par}", name=f"lwg{t}_{par}")
                    nc.sync.dma_start(out=lwg[key], in_=dr[f"lwg{t}_{par}"][:])
                    bias[key] = consts.tile([128, 1], f32,
                                            tag=f"bias{t}_{par}", name=f"bias{t}_{par}")
                    nc.sync.dma_start(out=bias[key], in_=dr[f"bias{t}_{par}"][:])
            iadda = consts.tile([96, 32], dt, tag="iadda", name="iadda")
            nc.sync.dma_start(out=iadda, in_=dr["iadda"][:])
            hw = {}
            for q in range(3):
                rows = 120 if q < 2 else 60
                for nm, ncol in (("wh1", 104), ("wh2", 60)):
                    key = f"{nm}_{q}"
                    hw[key] = consts.tile([rows, ncol], dt, tag=key, name=key)
                    nc.sync.dma_start(out=hw[key], in_=dr[key][:])
            for nm in ("w2full", "w2cde", "w3blk"):
                hw[nm] = consts.tile(list(dr[nm].shape), dt, tag=nm, name=nm)
                nc.sync.dma_start(out=hw[nm], in_=dr[nm][:])
            hb = consts.tile([128, 4], f32, tag="hbias")
            nc.sync.dma_start(out=hb, in_=dr["hbias"][:])

            # process tiles in octets
            for base in range(0, n_tiles, n_ilv):
                xs = list(range(base, base + n_ilv))
                col_of = {x: x * FD for x in xs}
                rhs = {}      # (x, t) -> tile
                S = {}
                CC = {}
                for x in xs:
                    for q in range(3):
                        rows = 120 if q < 2 else 60
                        CC[(x, q)] = ccp.tile([rows, FD], dt, tag=f"CC{x - base}_{q}",
                                              name=f"CC_{x}_{q}")

                def alloc_rhs(x, t):
                    if t > NSLOT:
                        return
                    if t == NSLOT:
                        rhs[(x, t)] = rhsp.tile([W3C, FD], dt, tag=f"rhs{x - base}",
                                                name=f"rhs_{x}_{t}")
                        return
                    rhs[(x, t)] = rhsp.tile([P[t]["Kmm"], FD], dt, tag=f"rhs{x - base}",
                                            name=f"rhs_{x}_{t}")

                def st_dma(x, t):
                    if t >= NSLOT:
                        return
                    info = P[t]
                    col = slice(col_of[x], col_of[x] + FD)
                    nc.sync.dma_start(out=rhs[(x, t)][info["st_lo"]:info["st_hi"], :],
                                      in_=dr[f"st{t}"][:, col])

                for x in xs:
                    alloc_rhs(x, 0)
                    alloc_rhs(x, 1)
                    st_dma(x, 0)

                for t in range(NSLOT):
                    info = P[t]
                    # prefetch next slot's state rows
                    for x in xs:
                        alloc_rhs(x, t + 1)
                    if t + 1 < NSLOT:
                        for x in xs:
                            st_dma(x, t + 1)
                    # late copy feeding slot t+1, issued before this slot's
                    # g' copy overwrites the source rows 0:32 of rhs[t]
                    if t + 1 in PLAN.late:
                        src_slot, src_row = PLAN.late[t + 1]
                        assert src_slot == t
                        lo = P[t + 1]["late_off"]
                        for x in xs:
                            nc.gpsimd.dma_start(
                                out=rhs[(x, t + 1)][lo:lo + 10, :],
                                in_=rhs[(x, t)][src_row:src_row + 10, :])
                    # gates matmul -> pt[0:128]; parity of x sets the M layout:
                    # even j: [ai|af|2ag|ao], odd j: [ai|af|ao|2ag]
                    pt = {}
                    for j, x in enumerate(xs):
                        pt[x] = pg.tile([128, FD], f32, tag="pt", name=f"pt_{x}_{t}")
                        for m in range(nchunk):
                            mc = slice(m * NMM, (m + 1) * NMM)
                            nc.tensor.matmul(pt[x][0:128, mc], lwg[(t, j % 2)][:],
                                             rhs[(x, t)][0:info["Kmm"], mc],
                                             start=True, stop=True)
                    # act1: sigma -> S
                    for j, x in enumerate(xs):
                        S[x] = sp.tile([128, FD], dt, tag=f"S{x - base}",
                                       name=f"S_{x}_{t}")
                        nc.scalar.activation(S[x][0:128, :], pt[x][0:128, :],
                                             AF.Sigmoid, bias=bias[(t, j % 2)][0:128])
                    # g' copy overwrites dead rhs rows 0:32 -> [g' | ct] at
                    # base 0 in one tile; U = [i*g' | f*c | i(copy)]
                    U = {}
                    for j, x in enumerate(xs):
                        gp_base = 64 if j % 2 == 0 else 96
                        nc.vector.tensor_copy(rhs[(x, t)][0:32, :],
                                              S[x][gp_base:gp_base + 32, :])
                        U[x] = up.tile([96, FD], dt, tag=f"U{x - base}",
                                       name=f"U_{x}_{t}")
                        nc.vector.tensor_mul(U[x][0:64, :], S[x][0:64, :],
                                             rhs[(x, t)][0:64, :])
                        nc.vector.tensor_copy(U[x][64:96, :], S[x][0:32, :])
                    # iadd (single mm) -> recycled pt[xb] rows 0:64;
                    # act2 = tanh(c2), shifted +64 so T2 lands at partner o's base
                    TP = {}
                    for qi in range(n_ilv // 2):
                        xa, xb = xs[2 * qi], xs[2 * qi + 1]   # even j, odd j
                        ptc = pt[xb]
                        for j2, x in ((1, xb), (0, xa)):      # odd -> rows 0:32
                            rb = 0 if j2 == 1 else 32
                            for m in range(nchunk):
                                mc = slice(m * NMM, (m + 1) * NMM)
                                nc.tensor.matmul(ptc[rb:rb + 32, mc], iadda[:],
                                                 U[x][0:96, mc], start=True, stop=True)
                        tp = s2p.tile([128, FD], dt, tag=f"S2{qi % 2}",
                                      name=f"S2_{base}_{t}_{qi}")
                        nc.scalar.activation(tp[64:128, :], ptc[0:64, :],
                                             AF.Tanh, scale=0.5)
                        TP[xb] = (tp, 64)     # odd tile: T2 @64, o @64
                        TP[xa] = (tp, 96)     # even tile: T2 @96, o @96
                    # h = tanh(c2) * o -> rhs[t+1][0:30]
                    for j, x in enumerate(xs):
                        tp, r0 = TP[x]
                        eng = nc.gpsimd if (j % 4 == 1) else nc.vector
                        eng.tensor_mul(rhs[(x, t + 1)][0:W3C, :],
                                       tp[r0:r0 + 30, :], S[x][r0:r0 + 30, :])
                    # concat copy (SWDGE on gpsimd)
                    ccq, rb = _cc_of(t)
                    for x in xs:
                        nc.gpsimd.dma_start(out=CC[(x, ccq)][rb:rb + 30, :],
                                            in_=rhs[(x, t + 1)][0:W3C, :])

                # ---- heads (per group of 3 to keep ring usage acyclic) ----
                FH, FH2 = {}, {}
                for qi, grp in enumerate(GROUPS):
                    quad = [xs[g] for g in grp]
                    for x in quad:
                        psA = pg.tile([128, FD], f32, tag="pt", name=f"psA_{x}")
                        for m in range(nchunk):
                            mc = slice(m * NMM, (m + 1) * NMM)
                            for q in range(3):
                                nc.tensor.matmul(psA[0:104, mc], hw[f"wh1_{q}"][:],
                                                 CC[(x, q)][:, mc],
                                                 start=(q == 0), stop=(q == 2))
                        FH[x] = fhp.tile([104, FD], dt, tag=f"FH{(x - base) % 2}",
                                         name=f"FH_{x}")
                        nc.scalar.activation(FH[x][0:104, :], psA[0:104, :],
                                             AF.Tanh, bias=hb[0:104, 0:1])
                        psB = pg.tile([128, FD], f32, tag="pt", name=f"psB_{x}")
                        for m in range(nchunk):
                            mc = slice(m * NMM, (m + 1) * NMM)
                            for q in range(3):
                                nc.tensor.matmul(psB[0:60, mc], hw[f"wh2_{q}"][:],
                                                 CC[(x, q)][:, mc],
                                                 start=(q == 0), stop=(q == 2))
                        FH2[x] = fhp.tile([60, FD], dt, tag=f"FH2{(x - base) % 2}",
                                          name=f"FH2_{x}")
                        nc.scalar.activation(FH2[x][0:60, :], psB[0:60, :],
                                             AF.Tanh, bias=hb[0:60, 1:2])
                    psF = pg.tile([128, FD], f32, tag="pt", name=f"psF_{base}_{qi}")
                    for j, x in enumerate(quad):
                        for m in range(nchunk):
                            mc = slice(m * NMM, (m + 1) * NMM)
                            nc.tensor.matmul(psF[32 * j:32 * j + 32, mc], hw["w2full"][:],
                                             FH[x][0:104, mc], start=True, stop=False)
                            nc.tensor.matmul(psF[32 * j:32 * j + 32, mc], hw["w2cde"][:],
                                             FH2[x][0:60, mc], start=False, stop=True)
                    nrF = 32 * (len(quad) - 1) + 10
                    F2q = fhp.tile([74, FD], dt, tag=f"F2{qi}", name=f"F2_{base}_{qi}")
                    nc.scalar.activation(F2q[0:nrF, :], psF[0:nrF, :],
                                         AF.Tanh, bias=hb[0:nrF, 2:3])
                    psO = pg.tile([128, FD], f32, tag="pt", name=f"psO_{base}_{qi}")
                    for m in range(nchunk):
                        mc = slice(m * NMM, (m + 1) * NMM)
                        nc.tensor.matmul(psO[0:len(quad), mc], hw["w3blk"][0:nrF, 0:len(quad)],
                                         F2q[0:nrF, mc], start=True, stop=True)
                    outq = outp.tile([3, FD], f32, tag=f"out{qi}", name=f"outq_{base}_{qi}")
                    nc.scalar.activation(outq[0:len(quad), :], psO[0:len(quad), :],
                                         AF.Tanh, bias=hb[0:len(quad), 3:4])
                    for j, x in enumerate(quad):
                        nc.gpsimd.dma_start(
                            out=out_d[0:1, col_of[x]:col_of[x] + FD],
                            in_=outq[j:j + 1, :])

    nc.finalize()
    return nc


def kernel(**inputs):
    import ml_dtypes
    np_dt = ml_dtypes.bfloat16
    FD = 1024
    inputs = {k: np.asarray(v) for k, v in inputs.items()}
    packed = pack_host(inputs, np_dt)
    nc = build_nc(BC, FD, np_dt)

    batch_keys = [k for k in packed if k.startswith(("st", "ct"))]
    in_maps = []
    for c in range(NCORE):
        m = {}
        for k, v in packed.items():
            if k in batch_keys:
                m[k] = np.ascontiguousarray(v[:, c * BC:(c + 1) * BC])
            else:
                m[k] = v
        in_maps.append(m)

    from concourse.bass_utils import run_bass_kernel_spmd
    res = run_bass_kernel_spmd(nc, in_maps, list(range(NCORE)))
    outs = [res.results[c]["out"].reshape(-1) for c in range(NCORE)]
    return np.concatenate(outs).reshape(B, 1).astype(np.float32)


if __name__ == "__main__":
    pass
